# revision 24
# baseline (speedup 1.0000x reference)
"""KAN layer (Catmull-Rom spline edges) as a single-matmul Trainium2 kernel.

Math:
  out[n,o] = sum_j w[o,j] * s_oj(x[n,j]) + bias[o],  s_oj = Catmull-Rom spline
  with K=8 uniform knots on [-1,1].  Each edge spline is decomposed into
  17 atom-chunks (near-side truncated-power basis; 5-tap stencils annihilate
  cubics so the decomposition is well-conditioned):
      out = sum_c  Acol_c^T @ H_c   + bias (folded into the PSUM->SBUF copy)
  H atoms: xc, xc^2, xc^3, z_s^2, z_s^3 (s=1..6), hD, hE
      z_s = min(xc - m'_s, 0) for s<=3, max(xc - m'_s, 0) for s>=4,
            m'_s = (s-3.5)/3.5
      hD = (xc+1)*z_1^2,  hE = z_6^2*(3.5*z_6 - 1)
  A-side (pure weight prepack) on host in fp16; atoms computed on device in
  fp16 (single-pass PE matmuls; rel-err ~3e-3 vs fp32 reference, gate 2e-2).
  Data-parallel over N across 8 NeuronCores.

Chunk order (acat columns AND matmul emission) matches atom readiness:
  [p1, p2, p3, s1, s2, s3, s4, s5, s6, D, c1, c2, c3, E, c4, c5, c6]
"""
import numpy as np
from math import comb

N, D_IN, D_OUT, K = 1024, 128, 128, 8
N_CORES = 8
N_LOC = N // N_CORES
N_CHUNKS = 15

_A_COEF = {-2: 0.5, -1: -2.0, 0: 3.0, 1: -2.0, 2: 0.5}
_B_COEF = {-2: -0.5, -1: 1.0, 0: 0.0, 1: -1.0, 2: 0.5}

# emission order [p1,p2,s1..s6,c1..c6,p3]: p3 last (its atom xc^3 is the
# slowest to produce; everything else streams while it finishes)
_PERM = [0, 1, 3, 4, 5, 6, 7, 8, 9, 10, 11, 12, 13, 14, 2]

_STATE = {}


def _poly_xc(s, p):
    """coeffs of (t-s)^p in powers of xc (const..xc^3), t = 3.5*xc + 3.5."""
    c = np.zeros(4)
    for i in range(p + 1):
        c[i] = comb(p, i) * (3.5 ** i) * ((3.5 - s) ** (p - i))
    return c


def _prepack(coeffs, weights, bias):
    """Host weight prepack -> (A_jm fp16 [j, 17*o] j-major, bias_col fp32 [o,1])."""
    Ap = (coeffs.astype(np.float64) * weights.astype(np.float64)[:, :, None]
          ).transpose(1, 2, 0)                                   # [j,k,o]
    poly = np.zeros((4, D_IN, D_OUT))
    cube = np.zeros((6, D_IN, D_OUT))
    sq = np.zeros((6, D_IN, D_OUT))
    for k in range(K):
        for r in (-2, -1, 0, 1, 2):
            s = k + r
            ar, br = _A_COEF[r], _B_COEF[r]
            if s >= 7:
                continue
            if s <= 3:
                # a(t-s)_+^3 + b(t-s)_+^2
                #   = [a(t-s)^3 + b(t-s)^2] + a*(s-t)_+^3 - b*(s-t)_+^2
                poly += (ar * _poly_xc(s, 3) + br * _poly_xc(s, 2)
                         )[:, None, None] * Ap[:, k, :][None]
                if s >= 1:
                    # z_s = min(.,0): (s-t)_+^3 = -42.875 z^3,
                    #                 (s-t)_+^2 =  12.25 z^2
                    cube[s - 1] += -42.875 * ar * Ap[:, k, :]
                    sq[s - 1] += -12.25 * br * Ap[:, k, :]
            else:
                # z_s = max(.,0): (t-s)_+^3 = 42.875 z^3, (t-s)_+^2 = 12.25 z^2
                cube[s - 1] += 42.875 * ar * Ap[:, k, :]
                sq[s - 1] += 12.25 * br * Ap[:, k, :]
    D_col = -21.4375 * Ap[:, 0, :]       # atom (xc+1)*z_1^2
    E_col = 6.125 * Ap[:, 7, :]          # atom z_6^2*(3.5 z_6 - 1)
    # Fold the boundary atoms into existing chunks (exact identities on the
    # truncated supports):  hD = (xc+1) z1^2 = z1^3 + (1+m'_1) z1^2
    #                       hE = z6^2 (3.5 z6 - 1) = 3.5 z6^3 - z6^2
    m1 = (1 - 3.5) / 3.5
    cube[0] += D_col
    sq[0] += (1.0 + m1) * D_col
    cube[5] += 3.5 * E_col
    sq[5] -= E_col
    A = np.stack([poly[1], poly[2], poly[3], sq[0], sq[1], sq[2],
                  sq[3], sq[4], sq[5],
                  cube[0], cube[1], cube[2], cube[3], cube[4], cube[5]]
                 )[_PERM]                                        # [15,j,o]
    A_jm = np.ascontiguousarray(
        A.transpose(1, 0, 2).reshape(D_IN, N_CHUNKS * D_OUT)
    ).astype(np.float16)                                         # [j,(c,o)]
    bias_row = (bias.astype(np.float64) + poly[0].sum(axis=0)
                ).astype(np.float16)[None, :]                    # [1,o]
    return A_jm, bias_row


# DMA groups over the (emitted) chunk axis: p1,p2 | squares | cubes+p3
_G_POLY = (0, 2)
_G_SQ = (2, 8)
_G_CUBE = (8, 15)


def _patch_sem_range():
    """Shrink the semaphore space: the walrus NEFF epilogue clears every
    semaphore in [7, max-sem-num) one-by-one (~130ns each, ~250 clears =
    ~7us of fixed teardown).  This kernel uses ~25 sems; remap the Bass
    kernel-sem range down from [150,256) to [24,256) and cap walrus at 48
    so the clear loop shrinks ~6x.  Runtime-used sems all stay < 48."""
    import concourse.bass as bass
    import concourse.bass_utils as bu

    # (Measured: remapping sems low + --max-sem-num does NOT shrink the
    # walrus clear loop, and concentrating sems in one bank slowed every
    # op's sem update. Keep stock behavior.)
    return


def _build_module():
    import concourse.bacc as bacc
    import concourse.bass as bass
    import concourse.mybir as mybir
    from concourse import tile

    _patch_sem_range()

    f32 = mybir.dt.float32
    f16 = mybir.dt.float16
    Alu = mybir.AluOpType
    Act = mybir.ActivationFunctionType
    ts = bass.ts

    # Skip the all-engine barrier Bass.__init__ emits after the const-AP
    # memsets (~0.5us before the first DMA issue can happen).  The only
    # const-AP readers here are ACT ops gated >2us later by input DMAs,
    # so the GPSIMD memsets always complete first.
    _orig_barrier = bass.Bass.all_engine_barrier

    def _skip_once(self, *a, **k):
        bass.Bass.all_engine_barrier = _orig_barrier
        return None

    bass.Bass.all_engine_barrier = _skip_once
    try:
        nc = bacc.Bacc("TRN2", target_bir_lowering=False, debug=False,
                       enable_asserts=False, num_devices=N_CORES)
    finally:
        bass.Bass.all_engine_barrier = _orig_barrier
    xt = nc.dram_tensor("xt", [D_IN, N_LOC], f32, kind="ExternalInput").ap()
    acat = nc.dram_tensor("acat", [D_IN, N_CHUNKS * D_OUT], f16,
                          kind="ExternalInput").ap()
    biasr = nc.dram_tensor("biasr", [1, D_OUT], f16, kind="ExternalInput").ap()
    out_t = nc.dram_tensor("out_t", [D_OUT, N_LOC], f32,
                           kind="ExternalOutput").ap()

    mprime = [(s - 3.5) / 3.5 for s in range(1, 7)]
    HB = 3 * N_LOC  # half-block of z columns

    with tile.TileContext(nc) as tc:
        with (
            tc.tile_pool(name="sbuf", bufs=1) as pool,
            tc.tile_pool(name="psum", bufs=1, space="PSUM") as ppool,
        ):
            x_sb = pool.tile([D_IN, N_LOC], f32, tag="x")
            a_sb = pool.tile([D_IN, N_CHUNKS * D_OUT], f16, tag="acat")
            b_sb = pool.tile([1, D_OUT], f16, tag="bias")

            # ---- DMAs first, spread across 3 trigger queues so the
            # transfers overlap: x + squares on Sync/HWDGE, polys + cubes
            # on the ACT queue, bias on Pool/SWDGE.
            nc.sync.dma_start(x_sb[:], xt[:])
            nc.scalar.dma_start(a_sb[:, _G_POLY[0] * D_OUT:_G_POLY[1] * D_OUT],
                                acat[:, _G_POLY[0] * D_OUT:_G_POLY[1] * D_OUT])
            nc.scalar.dma_start(a_sb[:, _G_CUBE[0] * D_OUT:_G_CUBE[1] * D_OUT],
                                acat[:, _G_CUBE[0] * D_OUT:_G_CUBE[1] * D_OUT])
            nc.sync.dma_start(a_sb[:, _G_SQ[0] * D_OUT:_G_SQ[1] * D_OUT],
                              acat[:, _G_SQ[0] * D_OUT:_G_SQ[1] * D_OUT])
            nc.gpsimd.dma_start(b_sb[:], biasr[:])

            # ones row for the rank-1 bias matmul; mb bias columns for the
            # ACT Relu shifts (const-AP registry only has 0.0/1.0)
            ones = pool.tile([1, N_LOC], f16, tag="ones")
            nc.gpsimd.memset(ones[:], 1.0)
            mb = pool.tile([D_IN, 2], f32, tag="mb")
            nc.gpsimd.memset(mb[:, 0:1], -mprime[3])
            nc.gpsimd.memset(mb[:, 1:2], -mprime[4])

            # ---- x-side atoms (fp16 after the clamp) ----
            # DVE: xcb, z1-3, z6, sq-half1, cube-halves, xc3
            # ACT: z4, z5 (relu-form), sq-half2;  GPSIMD: xc2
            xcb = pool.tile([D_IN, N_LOC], f16, tag="xcb")
            nc.vector.tensor_scalar(xcb[:], x_sb[:], -1.0, 1.0, Alu.max,
                                    Alu.min)

            z = pool.tile([D_IN, 6 * N_LOC], f16, tag="z")
            for i in range(3):
                nc.vector.tensor_scalar(z[:, ts(i, N_LOC)], xcb[:],
                                        mprime[i], 0.0, Alu.subtract, Alu.min)
            nc.vector.tensor_scalar(z[:, ts(5, N_LOC)], xcb[:],
                                    mprime[5], 0.0, Alu.subtract, Alu.max)

            xc2 = pool.tile([D_IN, N_LOC], f16, tag="xc2")
            nc.gpsimd.tensor_tensor(xc2[:], xcb[:], xcb[:], Alu.mult)
            for i in (3, 4):
                nc.scalar.activation(z[:, ts(i, N_LOC)], xcb[:], Act.Relu,
                                     bias=mb[:, i - 3:i - 2], scale=1.0)

            z2 = pool.tile([D_IN, 6 * N_LOC], f16, tag="z2")
            nc.vector.tensor_tensor(z2[:, 0:HB], z[:, 0:HB], z[:, 0:HB],
                                    Alu.mult)
            nc.scalar.activation(z2[:, HB:], z[:, HB:], Act.Square)

            z3 = pool.tile([D_IN, 6 * N_LOC], f16, tag="z3")
            nc.vector.tensor_tensor(z3[:, 0:HB], z2[:, 0:HB], z[:, 0:HB],
                                    Alu.mult)
            nc.vector.tensor_tensor(z3[:, HB:], z2[:, HB:], z[:, HB:],
                                    Alu.mult)
            xc3 = pool.tile([D_IN, N_LOC], f16, tag="xc3")
            nc.vector.tensor_tensor(xc3[:], xc2[:], xcb[:], Alu.mult)

            # ---- contraction: rank-1 bias + 15 accumulating fp16 matmuls
            # chunk order [p1,p2, s1..s6, c1..c6, p3]
            H = ([xcb[:], xc2[:]]
                 + [z2[:, ts(i, N_LOC)] for i in range(6)]
                 + [z3[:, ts(i, N_LOC)] for i in range(6)]
                 + [xc3[:]])
            psum = ppool.tile([D_OUT, N_LOC], f32, tag="acc")
            nc.tensor.matmul(psum[:], lhsT=b_sb[:], rhs=ones[:],
                             start=True, stop=False)
            for c in range(N_CHUNKS):
                nc.tensor.matmul(psum[:], lhsT=a_sb[:, ts(c, D_OUT)], rhs=H[c],
                                 start=False, stop=(c == N_CHUNKS - 1))

            out_sb = pool.tile([D_OUT, N_LOC], f32, tag="out")
            nc.scalar.copy(out_sb[:], psum[:])
            nc.sync.dma_start(out_t[:], out_sb[:])

    nc.compile()
    return nc


def _get_module():
    if "nc" not in _STATE:
        _STATE["nc"] = _build_module()
    return _STATE["nc"]


def _run(x, coeffs, weights, bias, trace=False, tmpdir=None):
    from concourse import bass_utils

    nc = _get_module()
    A_jm, bias_row = _prepack(coeffs, weights, bias)
    xT = np.ascontiguousarray(x.astype(np.float32).T)          # [j, N]
    in_maps = [
        {"xt": np.ascontiguousarray(xT[:, i * N_LOC:(i + 1) * N_LOC]),
         "acat": A_jm, "biasr": bias_row}
        for i in range(N_CORES)
    ]
    res = bass_utils.run_bass_kernel_spmd(
        nc, in_maps, core_ids=list(range(N_CORES)), trace=trace,
        tmpdir=tmpdir)
    out = np.concatenate([res.results[i]["out_t"] for i in range(N_CORES)],
                         axis=1).T                              # [N, o]
    return np.ascontiguousarray(out), res


def kernel(x, coeffs, weights, bias):
    out, _ = _run(np.asarray(x), np.asarray(coeffs), np.asarray(weights),
                  np.asarray(bias))
    return out
